# revision 54
# baseline (speedup 1.0000x reference)
"""HBV-2 hydrology model (nn_Hbv_2_5214090298013) as a Bass/Tile kernel on 8 NeuronCores.

Strategy: embarrassingly data-parallel across basins; each core gets 1250
basins laid out as [125 partitions, 10 basins] with nmul=2 components in
the free axis (f = m*10 + c). The 730-step recurrence runs as a fully
unrolled scan on DVE/ACT/POOL; forcing-derived arrays are precomputed per
73-step chunk on POOL; the 15-tap gamma unit-hydrograph conv runs at the
end split across DVE/POOL.

End-to-end (wall-clock) optimizations over the first working version:
 - All dynamic inputs ship as ONE fp16 tensor (halves tunnel bytes; fewer
   device_put round-trips). Engines upconvert fp16 operands exactly.
 - tmean is fp16 with a host-side one-ulp nudge so the rain/snow mask
   (Tt >= parTT) matches the f32 decision exactly — the only discontinuous
   use of a forcing input.
 - Static per-basin parameters (bound-scaled params, reciprocals, UH
   weights) are precomputed on host and ship as one small f32 tensor.
 - Output ships fp16 and is upconverted on host.
 - The PJRT executable is traced/jitted once and cached; later calls
   donate the previous call's output buffer as the (fully overwritten)
   output allocation, so no zero-buffer upload.
"""

import os
import sys

import numpy as np

for _p in ("/opt/trn_rl_repo",):
    if _p not in sys.path and os.path.isdir(_p):
        sys.path.insert(0, _p)

import concourse.bacc as bacc
import concourse.bass as bass
import concourse.mybir as mybir
from concourse.tile import TileContext

F32 = mybir.dt.float32
F16 = mybir.dt.float16
U16 = mybir.dt.uint16
U8 = mybir.dt.uint8
OP = mybir.AluOpType
AF = mybir.ActivationFunctionType

# Problem constants (hardcoded per contract)
T_TOTAL = int(os.environ.get("HBV_T", "730"))
N_GRID = 10000
NMUL = 2
NCORES = 8
GC = N_GRID // NCORES          # 1250 basins per core
P = 125                        # partitions used
C = GC // P                    # 10 basins per partition
F = NMUL * C                   # 20 state elems per partition
LENF = 15
NZ = 1e-5

TC = 73                        # time chunk
assert T_TOTAL % TC == 0
NCH = T_TOTAL // TC

# Input tensors, one per logical region so host prep pipelines with tunnel
# transfers (prep region k+1 overlaps region k's transfer). All ship as
# fixed point: u16 for prcp/pet/dy, 24-bit (u16 high + u8 low) for tmean —
# it feeds the discontinuous rain/snow mask (T >= parTT) and the melt
# terms, and u16's half-step there costs ~2e-2 max rel err. The mask is
# kept exactly consistent with the f32 reference decision by a host-side
# +-1 nudge of the 24-bit code (possible because parTT is per-basin static).
SCL_PRC = 10.0 / 65535.0
SCL_PET = 5.0 / 65535.0
SCL_DY = 1.0 / 65535.0
LO_TMP = -10.0
SCL_T24 = 30.0 / (65535.0 * 256.0)   # v24 = (T - LO_TMP) / SCL_T24

# cst (f32) regions: prescaled static arrays [F] each, then UH weights.
# TTlo = parTT - LO_TMP; TTu24 = parTT as an exact v24 mask threshold.
CST_ORDER = ["TTlo", "TTu24", "parCFMAX", "parCWH", "parFC", "invFC",
             "invLPFC", "ncc", "parC", "parPERC", "parUZL", "parK0", "parK1",
             "parK2", "rtclip"]
SIDX = {n: i for i, n in enumerate(CST_ORDER)}
NCST = len(CST_ORDER) * F + LENF * C

BOUNDS = {"parBETA": (1.0, 6.0), "parFC": (50.0, 1000.0), "parK0": (0.05, 0.9),
          "parK1": (0.01, 0.5), "parK2": (0.001, 0.2), "parLP": (0.2, 1.0),
          "parPERC": (0.0, 10.0), "parUZL": (0.0, 100.0), "parTT": (-2.5, 2.5),
          "parCFMAX": (0.5, 10.0), "parCFR": (0.0, 0.1), "parCWH": (0.0, 0.2),
          "parBETAET": (0.3, 5.0), "parC": (0.0, 1.0), "parRT": (0.0, 20.0),
          "parAC": (0.0, 2500.0)}
STAT_NAMES = ["parFC", "parK0", "parK1", "parK2", "parLP", "parPERC", "parUZL",
              "parTT", "parCFMAX", "parCFR", "parCWH", "parC", "parRT", "parAC"]
ROUT_A = (0.0, 2.9)
ROUT_B = (0.0, 6.5)

_TIMING = bool(os.environ.get("HBV_TIMING"))


def _build(nc: bass.Bass):
    T = T_TOTAL
    f32 = F32

    cstd = nc.dram_tensor("cst", [P, NCST], f32, kind="ExternalInput")
    prcd = nc.dram_tensor("prc", [P, T * C], U16, kind="ExternalInput")
    petd = nc.dram_tensor("pet", [P, T * C], U16, kind="ExternalInput")
    tmphd = nc.dram_tensor("tmph", [P, T * C], U16, kind="ExternalInput")
    tmpld = nc.dram_tensor("tmpl", [P, T * C], U8, kind="ExternalInput")
    dybd = nc.dram_tensor("dyb", [P, T * F], U16, kind="ExternalInput")
    dyed = nc.dram_tensor("dye", [P, T * F], U16, kind="ExternalInput")
    flowd = nc.dram_tensor("flow", [P, T * C], F16, kind="ExternalOutput")

    with TileContext(nc) as tc:
        with (
            tc.tile_pool(name="cst", bufs=1) as cst,
            tc.tile_pool(name="big", bufs=1) as big,
            tc.tile_pool(name="io", bufs=2) as iop,
            tc.tile_pool(name="der", bufs=2) as der,
        ):
            V = nc.vector
            G = nc.gpsimd
            A = nc.scalar

            cst_t = cst.tile([P, NCST], f32)
            nc.gpsimd.dma_start(out=cst_t[:, :], in_=cstd[:, :])

            def sp(name):
                i = SIDX[name]
                return cst_t[:, i * F:(i + 1) * F]

            def spm(name, m):
                i = SIDX[name]
                return cst_t[:, i * F + m * C: i * F + (m + 1) * C]

            NSTAT = len(CST_ORDER)

            def wnk(k):
                return cst_t[:, NSTAT * F + k * C: NSTAT * F + (k + 1) * C]

            # ---------------- states + big buffers ----------------
            SP_ = cst.tile([P, F], f32)
            MW = cst.tile([P, F], f32)
            SM = cst.tile([P, F], f32)
            SUZ = cst.tile([P, F], f32)
            SLZ = cst.tile([P, F], f32)
            for s in (SP_, MW, SM, SUZ, SLZ):
                V.memset(s[:, :], 1e-3)
            Qbuf = big.tile([P, T * C], F16)
            FLOW = big.tile([P, T * C], f32)

            # scratch tiles for the scan (persistent, reused every step)
            def scratch(name):
                tl = cst.tile([P, F], f32, tag=f"scr_{name}")
                return tl
            s_sp1 = scratch("sp1"); s_melt = scratch("melt"); s_mw1 = scratch("mw1")
            s_rfz = scratch("rfz"); s_mw2 = scratch("mw2"); s_cw = scratch("cw")
            s_t9 = scratch("t9"); s_tos = scratch("tos"); s_rts = scratch("rts")
            s_x = scratch("x"); s_lx = scratch("lx"); s_e = scratch("e")
            s_pw = scratch("pw"); s_rch = scratch("rch"); s_d1 = scratch("d1")
            s_sm1 = scratch("sm1"); s_sm2 = scratch("sm2"); s_exs = scratch("exs")
            s_y = scratch("y"); s_ly = scratch("ly"); s_f2 = scratch("f2")
            s_ev = scratch("ev"); s_pe = scratch("pe"); s_eta = scratch("eta")
            s_sm3 = scratch("sm3"); s_z = scratch("z"); s_zm = scratch("zm")
            s_u1 = scratch("u1"); s_cap = scratch("cap")
            s_su1 = scratch("su1"); s_su2 = scratch("su2"); s_suz2 = scratch("suz2")
            s_perc = scratch("perc"); s_q0a = scratch("q0a"); s_q0 = scratch("q0")
            s_q1 = scratch("q1"); s_gw2 = scratch("gw2"); s_q2 = scratch("q2")
            s_qa = scratch("qa")

            # ---------------- chunked main loop ----------------
            for ch in range(NCH):
                c0 = ch * TC
                prct = iop.tile([P, TC * C], U16, tag="prct")
                tmpht = iop.tile([P, TC * C], U16, tag="tmpht")
                tmplt = iop.tile([P, TC * C], U8, tag="tmplt")
                pett = iop.tile([P, TC * C], U16, tag="pett")
                dybt = iop.tile([P, TC * F], U16, tag="dybt")
                dyet = iop.tile([P, TC * F], U16, tag="dyet")
                nc.gpsimd.dma_start(out=prct[:, :],
                                    in_=prcd[:, c0 * C:(c0 + TC) * C])
                nc.gpsimd.dma_start(out=tmpht[:, :],
                                    in_=tmphd[:, c0 * C:(c0 + TC) * C])
                nc.gpsimd.dma_start(out=tmplt[:, :],
                                    in_=tmpld[:, c0 * C:(c0 + TC) * C])
                nc.gpsimd.dma_start(out=pett[:, :],
                                    in_=petd[:, c0 * C:(c0 + TC) * C])
                nc.gpsimd.dma_start(out=dybt[:, :],
                                    in_=dybd[:, c0 * F:(c0 + TC) * F])
                nc.gpsimd.dma_start(out=dyet[:, :],
                                    in_=dyed[:, c0 * F:(c0 + TC) * F])

                # ---- bulk derive on POOL/DVE ----
                raint = der.tile([P, TC * F], f32, tag="raint")
                snowt = der.tile([P, TC * F], f32, tag="snowt")
                mpt = der.tile([P, TC * F], f32, tag="mpt")
                rpt = der.tile([P, TC * F], f32, tag="rpt")
                m1t = der.tile([P, TC * F], f32, tag="m1t")
                dybf = der.tile([P, TC * F], f32, tag="dybf")
                dyef = der.tile([P, TC * F], f32, tag="dyef")
                comb = der.tile([P, TC * C], f32, tag="comb")

                def fb3(tile_ap):  # [P, TC*C] -> [P,TC,C]
                    return tile_ap.rearrange("p (t c) -> p t c", t=TC)

                def dv(tile_ap, m):  # [P, TC*F] -> m-slice [P,TC,C]
                    return tile_ap.rearrange(
                        "p (t m c) -> p t m c", t=TC, m=NMUL)[:, :, m, :]

                def sbcm(name, m):  # static m-slice -> bc [P,TC,C]
                    return spm(name, m).unsqueeze(1).broadcast_to([P, TC, C])

                # reassemble 24-bit tmean code: v24 = hi*256 + lo (exact f32)
                V.scalar_tensor_tensor(comb[:, :], tmpht[:, :], 256.0,
                                       tmplt[:, :], OP.mult, OP.add)
                # 2D sem-absorbers: 3D ops below may carry at most 1 wait
                V.tensor_copy(m1t[:, 0:1], comb[:, 0:1])
                V.tensor_copy(raint[:, 0:1], comb[:, 0:1])
                V.tensor_copy(snowt[:, 0:1], comb[:, 0:1])
                V.tensor_copy(mpt[:, 0:1], cst_t[:, 0:1])
                V.tensor_copy(rpt[:, 0:1], comb[:, 0:1])
                T3 = fb3(comb[:, :])
                P3 = fb3(prct[:, :])
                for m in range(NMUL):
                    # m1t = (v24*scl) - (parTT - LO) == T_degC - parTT
                    V.scalar_tensor_tensor(dv(m1t[:, :], m), T3, SCL_T24,
                                           sbcm("TTlo", m), OP.mult, OP.subtract)
                    # mask on the exact integer code (host-nudged)
                    V.tensor_tensor(dv(raint[:, :], m), T3,
                                    sbcm("TTu24", m), OP.is_ge)
                    # rain/snow kept in u16 prcp units; scaled at use sites
                    V.tensor_tensor(dv(raint[:, :], m), dv(raint[:, :], m),
                                    P3, OP.mult)
                    V.tensor_tensor(dv(snowt[:, :], m), P3,
                                    dv(raint[:, :], m), OP.subtract)
                for m in range(NMUL):
                    V.tensor_tensor(dv(mpt[:, :], m), dv(m1t[:, :], m),
                                    sbcm("parCFMAX", m), OP.mult)
                V.tensor_scalar_max(mpt[:, :], mpt[:, :], 0.0)
                V.tensor_scalar_min(m1t[:, :], m1t[:, :], 0.0)
                for m in range(NMUL):
                    V.tensor_tensor(dv(rpt[:, :], m), dv(m1t[:, :], m),
                                    sbcm("ncc", m), OP.mult)
                # dequant + scale dynamic params u16 -> f32
                V.tensor_scalar(dybf[:, :], dybt[:, :], 5.0 * SCL_DY, 1.0,
                                OP.mult, OP.add)
                V.tensor_scalar(dyef[:, :], dyet[:, :], 4.7 * SCL_DY, 0.3,
                                OP.mult, OP.add)

                # ---- sequential scan ----
                for t in range(TC):
                    SNOW_t = snowt[:, t * F:(t + 1) * F]
                    mp_t = mpt[:, t * F:(t + 1) * F]
                    rp_t = rpt[:, t * F:(t + 1) * F]
                    RAIN_t = raint[:, t * F:(t + 1) * F]
                    beta_t = dybf[:, t * F:(t + 1) * F]
                    betaet_t = dyef[:, t * F:(t + 1) * F]

                    # snow bucket (SNOW_t/RAIN_t are in u16 prcp units)
                    V.scalar_tensor_tensor(s_sp1[:, :], SNOW_t, SCL_PRC,
                                           SP_[:, :], OP.mult, OP.add)
                    V.tensor_tensor(s_melt[:, :], mp_t, s_sp1[:, :], OP.min)
                    V.tensor_tensor(s_mw1[:, :], MW[:, :], s_melt[:, :], OP.add)
                    V.tensor_tensor(s_sp1[:, :], s_sp1[:, :], s_melt[:, :], OP.subtract)
                    V.tensor_tensor(s_rfz[:, :], rp_t, s_mw1[:, :], OP.min)
                    V.tensor_tensor(SP_[:, :], s_sp1[:, :], s_rfz[:, :], OP.add)
                    V.tensor_tensor(s_mw2[:, :], s_mw1[:, :], s_rfz[:, :], OP.subtract)
                    V.tensor_tensor(s_cw[:, :], sp("parCWH"), SP_[:, :], OP.mult)
                    V.tensor_tensor(s_t9[:, :], s_mw2[:, :], s_cw[:, :], OP.subtract)
                    A.activation(s_tos[:, :], s_t9[:, :], AF.Relu)
                    V.tensor_tensor(MW[:, :], s_mw2[:, :], s_tos[:, :], OP.subtract)
                    V.scalar_tensor_tensor(s_rts[:, :], RAIN_t, SCL_PRC,
                                           s_tos[:, :], OP.mult, OP.add)

                    # soil bucket
                    V.tensor_tensor(s_x[:, :], SM[:, :], sp("invFC"), OP.mult)
                    A.activation(s_lx[:, :], s_x[:, :], AF.Ln)
                    V.tensor_tensor(s_e[:, :], beta_t, s_lx[:, :], OP.mult)
                    V.tensor_scalar_min(s_e[:, :], s_e[:, :], 0.0)
                    A.activation(s_pw[:, :], s_e[:, :], AF.Exp)
                    V.tensor_tensor(s_rch[:, :], s_rts[:, :], s_pw[:, :], OP.mult)
                    V.tensor_tensor(s_d1[:, :], s_rts[:, :], s_rch[:, :], OP.subtract)
                    V.tensor_tensor(s_sm1[:, :], SM[:, :], s_d1[:, :], OP.add)
                    V.tensor_tensor(s_sm2[:, :], s_sm1[:, :], sp("parFC"), OP.min)
                    V.tensor_tensor(s_exs[:, :], s_sm1[:, :], s_sm2[:, :], OP.subtract)
                    V.tensor_tensor(s_y[:, :], s_sm2[:, :], sp("invLPFC"), OP.mult)
                    A.activation(s_ly[:, :], s_y[:, :], AF.Ln)
                    V.scalar_tensor_tensor(s_f2[:, :], s_ly[:, :], 0.0,
                                           betaet_t, OP.min, OP.mult)
                    A.activation(s_ev[:, :], s_f2[:, :], AF.Exp)
                    for m in range(NMUL):
                        V.scalar_tensor_tensor(s_pe[:, m * C:(m + 1) * C],
                                               pett[:, t * C:(t + 1) * C],
                                               SCL_PET,
                                               s_ev[:, m * C:(m + 1) * C],
                                               OP.mult, OP.mult)
                    V.tensor_tensor(s_eta[:, :], s_sm2[:, :], s_pe[:, :], OP.min)
                    V.tensor_tensor(s_sm3[:, :], s_sm2[:, :], s_eta[:, :], OP.subtract)
                    V.tensor_scalar_max(s_sm3[:, :], s_sm3[:, :], NZ)
                    # capillary
                    V.tensor_tensor(s_z[:, :], s_sm3[:, :], sp("invFC"), OP.mult)
                    V.tensor_scalar(s_zm[:, :], s_z[:, :], 1.0, -1.0, OP.min, OP.mult)
                    V.tensor_tensor(s_u1[:, :], SLZ[:, :], sp("parC"), OP.mult)
                    V.scalar_tensor_tensor(s_cap[:, :], s_zm[:, :], 1.0,
                                           s_u1[:, :], OP.add, OP.mult)
                    V.tensor_tensor(SM[:, :], s_sm3[:, :], s_cap[:, :], OP.add)
                    V.tensor_tensor(SLZ[:, :], SLZ[:, :], s_cap[:, :], OP.subtract)
                    V.tensor_scalar_max(SLZ[:, :], SLZ[:, :], NZ)

                    # groundwater
                    G.tensor_tensor(s_su1[:, :], SUZ[:, :], s_rch[:, :], OP.add)
                    G.tensor_tensor(s_su1[:, :], s_su1[:, :], s_exs[:, :], OP.add)
                    G.tensor_tensor(s_su2[:, :], s_su1[:, :], sp("parPERC"), OP.subtract)
                    A.activation(s_suz2[:, :], s_su2[:, :], AF.Relu)
                    G.tensor_tensor(s_perc[:, :], s_su1[:, :], s_suz2[:, :], OP.subtract)
                    G.tensor_tensor(s_q0a[:, :], s_suz2[:, :], sp("parUZL"), OP.subtract)
                    V.scalar_tensor_tensor(s_q0[:, :], s_q0a[:, :], 0.0,
                                           sp("parK0"), OP.max, OP.mult)
                    G.tensor_tensor(s_suz2[:, :], s_suz2[:, :], s_q0[:, :], OP.subtract)
                    G.tensor_tensor(s_q1[:, :], sp("parK1"), s_suz2[:, :], OP.mult)
                    G.tensor_tensor(SUZ[:, :], s_suz2[:, :], s_q1[:, :], OP.subtract)
                    G.tensor_tensor(SLZ[:, :], SLZ[:, :], s_perc[:, :], OP.add)
                    G.tensor_tensor(s_gw2[:, :], SLZ[:, :], sp("rtclip"), OP.subtract)
                    V.scalar_tensor_tensor(s_q2[:, :], s_gw2[:, :], 0.0,
                                           sp("parK2"), OP.max, OP.mult)
                    V.scalar_tensor_tensor(SLZ[:, :], s_gw2[:, :], 0.0,
                                           s_q2[:, :], OP.max, OP.subtract)
                    # Qt and nmul-sum (mean folded into UH weights)
                    G.tensor_tensor(s_qa[:, :], s_q0[:, :], s_q1[:, :], OP.add)
                    G.tensor_tensor(s_qa[:, :], s_qa[:, :], s_q2[:, :], OP.add)
                    tq = c0 + t
                    G.tensor_tensor(Qbuf[:, tq * C:(tq + 1) * C],
                                    s_qa[:, 0:C], s_qa[:, C:F], OP.add)

            # ---------------- UH routing ----------------
            # flow[t] = sum_k wn[k] * Q[t-k]; DVE handles t in [0,TS), POOL the rest
            TS = (T * 7) // 10
            rtmp = big.tile([P, T * C], f32)
            flow16 = big.tile([P, T * C], F16)

            def conv_range(eng, t_lo, t_hi):
                for k in range(LENF):
                    o_lo = max(t_lo, k)
                    n = t_hi - o_lo
                    if n <= 0:
                        continue
                    wk_bc = wnk(k).unsqueeze(1).broadcast_to([P, n, C])
                    qsh = Qbuf[:, (o_lo - k) * C:(o_lo - k + n) * C] \
                        .rearrange("p (t c) -> p t c", t=n)
                    out = FLOW[:, o_lo * C:(o_lo + n) * C] \
                        .rearrange("p (t c) -> p t c", t=n)
                    if k == 0:
                        eng.tensor_tensor(out, wk_bc, qsh, OP.mult)
                    else:
                        tmp = rtmp[:, o_lo * C:(o_lo + n) * C] \
                            .rearrange("p (t c) -> p t c", t=n)
                        eng.tensor_tensor(tmp, wk_bc, qsh, OP.mult)
                        eng.tensor_tensor(out, out, tmp, OP.add)

            conv_range(V, 0, TS)
            conv_range(G, TS, T)
            # convert to fp16 for the wire, split across engines
            TH = T // 2
            V.tensor_copy(flow16[:, :TH * C], FLOW[:, :TH * C])
            G.tensor_copy(flow16[:, TH * C:], FLOW[:, TH * C:])

            nc.gpsimd.dma_start(out=flowd[:, :], in_=flow16[:, :])
    return nc


def _prep_forc_u16(src_TG, inv_scl, out_u16, scr):
    # round-to-nearest u16 quantization (float->uint assignment truncates,
    # so add 0.5 first) fused with the [T,G] -> [KP, T*C] layout transpose
    T, K = T_TOTAL, NCORES
    np.multiply(src_TG, np.float32(inv_scl), out=scr)
    scr += np.float32(0.5)
    out_u16.reshape(K, P, T, C)[...] = scr.reshape(T, K, P, C).transpose(1, 2, 0, 3)


def _tt_of(params_stat):
    st = params_stat[:, :14 * NMUL].reshape(N_GRID, 14, NMUL)
    return st[:, 7] * np.float32(5.0) + np.float32(-2.5)      # [G, NMUL]


def _prep_tmp24(x_phy, TT, tmph, tmpl, scr, scrh):
    """24-bit tmean code v24, nudged so (v24 >= TTu24) == (T32 >= parTT)
    exactly; split into u16 high and u8 low halves. All v24 values stay
    below 2^24, so every quantity here is exact in f32."""
    T, K = T_TOTAL, NCORES
    t32 = x_phy[:, :, 1]
    np.subtract(t32, np.float32(LO_TMP), out=scr)
    scr *= np.float32(1.0 / 30.0 * 65535.0 * 256.0)
    scr += np.float32(0.5)
    np.floor(scr, out=scr)                                    # v24
    TTu24 = (TT - np.float32(LO_TMP)) * np.float32(1.0 / SCL_T24)
    for _ in range(4):
        bad_up = None
        bad_dn = None
        for m in range(NMUL):
            m32 = t32 >= TT[None, :, m]
            mq = scr >= TTu24[None, :, m]
            up = m32 & ~mq
            dn = ~m32 & mq
            bad_up = up if bad_up is None else (bad_up | up)
            bad_dn = dn if bad_dn is None else (bad_dn | dn)
        if not (bad_up.any() or bad_dn.any()):
            break
        scr[bad_up] += np.float32(1.0)
        scr[bad_dn] -= np.float32(1.0)
    np.multiply(scr, np.float32(1.0 / 256.0), out=scrh)
    np.floor(scrh, out=scrh)                                  # hi
    tmph.reshape(K, P, T, C)[...] = scrh.reshape(T, K, P, C).transpose(1, 2, 0, 3)
    scrh *= np.float32(-256.0)
    scr += scrh                                               # lo = v24 - hi*256
    tmpl.reshape(K, P, T, C)[...] = scr.reshape(T, K, P, C).transpose(1, 2, 0, 3)


def _prep_dy(params_dy, j, out_u16, scr2):
    # dynamic params: u16 in [0,1]; j = 0 (parBETA) or 1 (parBETAET)
    T, K = T_TOTAL, NCORES
    d = params_dy.reshape(T, N_GRID, 2, NMUL)
    np.multiply(d[:, :, j, :], np.float32(65535.0), out=scr2)
    scr2 += np.float32(0.5)
    out_u16.reshape(K, P, T, NMUL, C)[...] = \
        scr2.reshape(T, K, P, C, NMUL).transpose(1, 2, 0, 4, 3)


def _prep_cst(ac_all, params_stat, TT, cst):
    G, K = N_GRID, NCORES
    st = params_stat[:, :14 * NMUL].reshape(G, 14, NMUL)

    # statics, prescaled in f32 (matches on-device math of the baseline)
    vals = {}
    for i, name in enumerate(STAT_NAMES):
        lo, hi = BOUNDS[name]
        vals[name] = st[:, i] * np.float32(hi - lo) + np.float32(lo)
    invFC = np.float32(1.0) / vals["parFC"]
    invLPFC = np.float32(1.0) / (vals["parLP"] * vals["parFC"])
    ncc = -(vals["parCFR"] * vals["parCFMAX"])
    acq = np.clip(np.float32(1.0) - ac_all[:, None].astype(np.float32)
                  / (vals["parAC"] + np.float32(NZ)), 0.0, 1.0).astype(np.float32)
    rtclip = vals["parRT"] * acq
    table = {"TTlo": TT - np.float32(LO_TMP),
             "TTu24": (TT - np.float32(LO_TMP)) * np.float32(1.0 / SCL_T24),
             "parCFMAX": vals["parCFMAX"], "parCWH": vals["parCWH"],
             "parFC": vals["parFC"], "invFC": invFC, "invLPFC": invLPFC,
             "ncc": ncc, "parC": vals["parC"], "parPERC": vals["parPERC"],
             "parUZL": vals["parUZL"], "parK0": vals["parK0"],
             "parK1": vals["parK1"], "parK2": vals["parK2"], "rtclip": rtclip}
    for name in CST_ORDER:
        i = SIDX[name]
        cst[:, i * F:(i + 1) * F].reshape(K, P, NMUL, C)[...] = \
            table[name].reshape(K, P, C, NMUL).transpose(0, 1, 3, 2)

    # UH weights (gammaln and theta^-a cancel under normalization);
    # fold the nmul-mean (x0.5) in
    rta = params_stat[:, 14 * NMUL].astype(np.float32)
    rtb = params_stat[:, 14 * NMUL + 1].astype(np.float32)
    a = np.maximum(rta * np.float32(ROUT_A[1]), 0) + np.float32(0.1)
    th = np.maximum(rtb * np.float32(ROUT_B[1]), 0) + np.float32(0.5)
    tk = (np.arange(LENF) + 0.5).astype(np.float32)
    w = np.exp((a - np.float32(1.0))[None, :] * np.log(tk)[:, None]
               - tk[:, None] / th[None, :]).astype(np.float32)
    w /= w.sum(0)
    w *= np.float32(0.5)
    cst[:, len(CST_ORDER) * F:].reshape(K, P, LENF, C)[...] = \
        w.reshape(LENF, K, P, C).transpose(1, 2, 0, 3)


_RT = {}


def _get_rt():
    if _RT:
        return _RT
    import jax
    from jax.sharding import Mesh, PartitionSpec, NamedSharding

    nc = bacc.Bacc()
    _build(nc)
    nc.compile()

    from concourse import bass2jax
    bass2jax.install_neuronx_cc_hook()

    partition_name = nc.partition_id_tensor.name if nc.partition_id_tensor else None
    in_names, out_names, out_avals = [], [], []
    for alloc in nc.m.functions[0].allocations:
        if not isinstance(alloc, mybir.MemoryLocationSet):
            continue
        name = alloc.memorylocations[0].name
        if alloc.kind == "ExternalInput":
            if name != partition_name:
                in_names.append(name)
        elif alloc.kind == "ExternalOutput":
            out_names.append(name)
            out_avals.append(jax.core.ShapedArray(
                tuple(alloc.tensor_shape), mybir.dt.np(alloc.dtype)))
    assert in_names == ["cst", "prc", "pet", "tmph", "tmpl", "dyb", "dye"] \
        and out_names == ["flow"], (in_names, out_names)
    n_params = len(in_names)
    in_names_all = in_names + out_names
    if partition_name is not None:
        in_names_all = in_names_all + [partition_name]

    def _body(*args):
        operands = list(args)
        if partition_name is not None:
            operands.append(bass2jax.partition_id_tensor())
        outs = bass2jax._bass_exec_p.bind(
            *operands, out_avals=tuple(out_avals), in_names=tuple(in_names_all),
            out_names=tuple(out_names), lowering_input_output_aliases=(),
            sim_require_finite=True, sim_require_nnan=True, nc=nc)
        return tuple(outs)

    devices = jax.devices()[:NCORES]
    assert len(devices) == NCORES
    mesh = Mesh(np.asarray(devices), ("core",))
    spec = PartitionSpec("core")
    fn = jax.jit(
        jax.shard_map(_body, mesh=mesh, in_specs=(spec,) * (n_params + 1),
                      out_specs=(spec,)),
        donate_argnums=(n_params,), keep_unused=True)
    from concurrent.futures import ThreadPoolExecutor
    _RT.update(dict(jax=jax, fn=fn, sh=NamedSharding(mesh, spec),
                    donate=None, pool=ThreadPoolExecutor(max_workers=8)))
    return _RT


def kernel(x_phy, ac_all, elev_all, params_dy, params_stat, _trace=False):
    import time
    t0 = time.time()
    rt = _get_rt()
    jax = rt["jax"]
    x_phy = np.asarray(x_phy, dtype=np.float32)
    ac_all = np.asarray(ac_all, dtype=np.float32)
    params_dy = np.asarray(params_dy, dtype=np.float32)
    params_stat = np.asarray(params_stat, dtype=np.float32)
    T = x_phy.shape[0]
    assert T == T_TOTAL, f"kernel built for T={T_TOTAL}, got {T}"
    t1 = time.time()

    sh = rt["sh"]
    bufs = rt.get("bufs")
    if bufs is None:
        KP = NCORES * P
        bufs = {"prc": np.empty((KP, T * C), np.uint16),
                "pet": np.empty((KP, T * C), np.uint16),
                "tmph": np.empty((KP, T * C), np.uint16),
                "tmpl": np.empty((KP, T * C), np.uint8),
                "dyb": np.empty((KP, T * F), np.uint16),
                "dye": np.empty((KP, T * F), np.uint16),
                "cst": np.empty((KP, NCST), np.float32),
                "scr": np.empty((T, N_GRID), np.float32),
                "scrh": np.empty((T, N_GRID), np.float32),
                "scr2": np.empty((T, N_GRID, NMUL), np.float32)}
        rt["bufs"] = bufs

    # staged prep -> put. Puts are issued from worker threads: device_put
    # can block when too many transfers are outstanding, and a blocked put
    # must not stall the main thread's prep of the remaining regions.
    pool = rt["pool"]
    futs = {}
    TT = _tt_of(params_stat)
    _prep_forc_u16(x_phy[:, :, 0], 1.0 / SCL_PRC, bufs["prc"], bufs["scr"])
    futs["prc"] = pool.submit(jax.device_put, bufs["prc"], sh)
    _prep_forc_u16(x_phy[:, :, 2], 1.0 / SCL_PET, bufs["pet"], bufs["scr"])
    futs["pet"] = pool.submit(jax.device_put, bufs["pet"], sh)
    _prep_tmp24(x_phy, TT, bufs["tmph"], bufs["tmpl"], bufs["scr"], bufs["scrh"])
    futs["tmph"] = pool.submit(jax.device_put, bufs["tmph"], sh)
    futs["tmpl"] = pool.submit(jax.device_put, bufs["tmpl"], sh)
    _prep_dy(params_dy, 0, bufs["dyb"], bufs["scr2"])
    futs["dyb"] = pool.submit(jax.device_put, bufs["dyb"], sh)
    _prep_dy(params_dy, 1, bufs["dye"], bufs["scr2"])
    futs["dye"] = pool.submit(jax.device_put, bufs["dye"], sh)
    _prep_cst(ac_all, params_stat, TT, bufs["cst"])
    futs["cst"] = pool.submit(jax.device_put, bufs["cst"], sh)
    don = rt["donate"]
    if don is None:
        don = jax.device_put(np.zeros((NCORES * P, T * C), np.float16), sh)
    t3 = time.time()

    out, = rt["fn"](futs["cst"].result(), futs["prc"].result(),
                    futs["pet"].result(), futs["tmph"].result(),
                    futs["tmpl"].result(), futs["dyb"].result(),
                    futs["dye"].result(), don)
    # per-shard fetch in worker threads, with the layout transpose for shard
    # k overlapping the tunnel fetch of shards k+1..: hides most of post
    out.block_until_ready()
    t4 = time.time()
    full = np.empty((T, N_GRID), np.float32)
    fullv = full.reshape(T, NCORES, P, C)
    shards = out.addressable_shards

    def _fetch(i):
        sh_ = shards[i]
        return (sh_.index[0].start or 0) // P, np.asarray(sh_.data)

    from concurrent.futures import as_completed
    fetch_futs = [pool.submit(_fetch, i) for i in range(len(shards))]
    for fu in as_completed(fetch_futs):
        k, arr = fu.result()
        fullv[:, k] = arr.reshape(P, T, C).transpose(1, 0, 2)
    rt["donate"] = out             # reuse device buffer as next call's output alloc
    t5 = time.time()
    if _TIMING:
        print(f"[kernel] setup {t1-t0:.3f}s prep+put {t3-t1:.3f}s "
              f"xfer+exec {t4-t3:.3f}s fetch+post {t5-t4:.3f}s total {t5-t0:.3f}s",
              flush=True)
    return full[..., None]


# revision 56
# speedup vs baseline: 1.2665x; 1.2665x over previous
"""HBV-2 hydrology model (nn_Hbv_2_5214090298013) as a Bass/Tile kernel on 8 NeuronCores.

Strategy: embarrassingly data-parallel across basins; each core gets 1250
basins laid out as [125 partitions, 10 basins] with nmul=2 components in
the free axis (f = m*10 + c). The 730-step recurrence runs as a fully
unrolled scan on DVE/ACT/POOL; forcing-derived arrays are precomputed per
73-step chunk on POOL; the 15-tap gamma unit-hydrograph conv runs at the
end split across DVE/POOL.

End-to-end (wall-clock) optimizations over the first working version:
 - All dynamic inputs ship as ONE fp16 tensor (halves tunnel bytes; fewer
   device_put round-trips). Engines upconvert fp16 operands exactly.
 - tmean is fp16 with a host-side one-ulp nudge so the rain/snow mask
   (Tt >= parTT) matches the f32 decision exactly — the only discontinuous
   use of a forcing input.
 - Static per-basin parameters (bound-scaled params, reciprocals, UH
   weights) are precomputed on host and ship as one small f32 tensor.
 - Output ships fp16 and is upconverted on host.
 - The PJRT executable is traced/jitted once and cached; later calls
   donate the previous call's output buffer as the (fully overwritten)
   output allocation, so no zero-buffer upload.
"""

import os
import sys

import numpy as np

for _p in ("/opt/trn_rl_repo",):
    if _p not in sys.path and os.path.isdir(_p):
        sys.path.insert(0, _p)

import concourse.bacc as bacc
import concourse.bass as bass
import concourse.mybir as mybir
from concourse.tile import TileContext

F32 = mybir.dt.float32
F16 = mybir.dt.float16
U16 = mybir.dt.uint16
U8 = mybir.dt.uint8
OP = mybir.AluOpType
AF = mybir.ActivationFunctionType

# Problem constants (hardcoded per contract)
T_TOTAL = int(os.environ.get("HBV_T", "730"))
N_GRID = 10000
NMUL = 2
NCORES = 8
GC = N_GRID // NCORES          # 1250 basins per core
P = 125                        # partitions used
C = GC // P                    # 10 basins per partition
F = NMUL * C                   # 20 state elems per partition
LENF = 15
NZ = 1e-5

TC = 73                        # time chunk
assert T_TOTAL % TC == 0
NCH = T_TOTAL // TC

# Input tensors, one per logical region so host prep pipelines with tunnel
# transfers (prep region k+1 overlaps region k's transfer). All ship as
# fixed point: u16 for prcp/pet/dy, 24-bit (u16 high + u8 low) for tmean —
# it feeds the discontinuous rain/snow mask (T >= parTT) and the melt
# terms, and u16's half-step there costs ~2e-2 max rel err. The mask is
# kept exactly consistent with the f32 reference decision by a host-side
# +-1 nudge of the 24-bit code (possible because parTT is per-basin static).
SCL_PRC = 10.0 / 65535.0
SCL_PET = 5.0 / 65535.0
SCL_DY = 1.0 / 65535.0
LO_TMP = -10.0
SCL_T24 = 30.0 / (65535.0 * 256.0)   # v24 = (T - LO_TMP) / SCL_T24

# cst (f32) regions: prescaled static arrays [F] each, then UH weights.
# TTlo = parTT - LO_TMP; TTu24 = parTT as an exact v24 mask threshold.
CST_ORDER = ["TTlo", "TTu24", "parCFMAX", "parCWH", "parFC", "invFC",
             "invLPFC", "ncc", "parC", "parPERC", "parUZL", "parK0", "parK1",
             "parK2", "rtclip"]
SIDX = {n: i for i, n in enumerate(CST_ORDER)}
NCST = len(CST_ORDER) * F + LENF * C

BOUNDS = {"parBETA": (1.0, 6.0), "parFC": (50.0, 1000.0), "parK0": (0.05, 0.9),
          "parK1": (0.01, 0.5), "parK2": (0.001, 0.2), "parLP": (0.2, 1.0),
          "parPERC": (0.0, 10.0), "parUZL": (0.0, 100.0), "parTT": (-2.5, 2.5),
          "parCFMAX": (0.5, 10.0), "parCFR": (0.0, 0.1), "parCWH": (0.0, 0.2),
          "parBETAET": (0.3, 5.0), "parC": (0.0, 1.0), "parRT": (0.0, 20.0),
          "parAC": (0.0, 2500.0)}
STAT_NAMES = ["parFC", "parK0", "parK1", "parK2", "parLP", "parPERC", "parUZL",
              "parTT", "parCFMAX", "parCFR", "parCWH", "parC", "parRT", "parAC"]
ROUT_A = (0.0, 2.9)
ROUT_B = (0.0, 6.5)

_TIMING = bool(os.environ.get("HBV_TIMING"))


def _build(nc: bass.Bass):
    T = T_TOTAL
    f32 = F32

    cstd = nc.dram_tensor("cst", [P, NCST], f32, kind="ExternalInput")
    prcd = nc.dram_tensor("prc", [P, T * C], U16, kind="ExternalInput")
    petd = nc.dram_tensor("pet", [P, T * C], U16, kind="ExternalInput")
    tmphd = nc.dram_tensor("tmph", [P, T * C], U16, kind="ExternalInput")
    tmpld = nc.dram_tensor("tmpl", [P, T * C], U8, kind="ExternalInput")
    dybd = nc.dram_tensor("dyb", [P, T * F], U16, kind="ExternalInput")
    dyed = nc.dram_tensor("dye", [P, T * F], U16, kind="ExternalInput")
    flowd = nc.dram_tensor("flow", [P, T * C], F16, kind="ExternalOutput")

    with TileContext(nc) as tc:
        with (
            tc.tile_pool(name="cst", bufs=1) as cst,
            tc.tile_pool(name="big", bufs=1) as big,
            tc.tile_pool(name="io", bufs=2) as iop,
            tc.tile_pool(name="der", bufs=2) as der,
        ):
            V = nc.vector
            G = nc.gpsimd
            A = nc.scalar

            cst_t = cst.tile([P, NCST], f32)
            nc.gpsimd.dma_start(out=cst_t[:, :], in_=cstd[:, :])

            def sp(name):
                i = SIDX[name]
                return cst_t[:, i * F:(i + 1) * F]

            def spm(name, m):
                i = SIDX[name]
                return cst_t[:, i * F + m * C: i * F + (m + 1) * C]

            NSTAT = len(CST_ORDER)

            def wnk(k):
                return cst_t[:, NSTAT * F + k * C: NSTAT * F + (k + 1) * C]

            # ---------------- states + big buffers ----------------
            SP_ = cst.tile([P, F], f32)
            MW = cst.tile([P, F], f32)
            SM = cst.tile([P, F], f32)
            SUZ = cst.tile([P, F], f32)
            SLZ = cst.tile([P, F], f32)
            for s in (SP_, MW, SM, SUZ, SLZ):
                V.memset(s[:, :], 1e-3)
            Qbuf = big.tile([P, T * C], F16)
            FLOW = big.tile([P, T * C], f32)

            # scratch tiles for the scan (persistent, reused every step)
            def scratch(name):
                tl = cst.tile([P, F], f32, tag=f"scr_{name}")
                return tl
            s_sp1 = scratch("sp1"); s_melt = scratch("melt"); s_mw1 = scratch("mw1")
            s_rfz = scratch("rfz"); s_mw2 = scratch("mw2"); s_cw = scratch("cw")
            s_t9 = scratch("t9"); s_tos = scratch("tos"); s_rts = scratch("rts")
            s_x = scratch("x"); s_lx = scratch("lx"); s_e = scratch("e")
            s_pw = scratch("pw"); s_rch = scratch("rch"); s_d1 = scratch("d1")
            s_sm1 = scratch("sm1"); s_sm2 = scratch("sm2"); s_exs = scratch("exs")
            s_y = scratch("y"); s_ly = scratch("ly"); s_f2 = scratch("f2")
            s_ev = scratch("ev"); s_pe = scratch("pe"); s_eta = scratch("eta")
            s_sm3 = scratch("sm3"); s_z = scratch("z"); s_zm = scratch("zm")
            s_u1 = scratch("u1"); s_cap = scratch("cap")
            s_su1 = scratch("su1"); s_su2 = scratch("su2"); s_suz2 = scratch("suz2")
            s_perc = scratch("perc"); s_q0a = scratch("q0a"); s_q0 = scratch("q0")
            s_q1 = scratch("q1"); s_gw2 = scratch("gw2"); s_q2 = scratch("q2")
            s_qa = scratch("qa")

            # ---------------- chunked main loop ----------------
            for ch in range(NCH):
                c0 = ch * TC
                prct = iop.tile([P, TC * C], U16, tag="prct")
                tmpht = iop.tile([P, TC * C], U16, tag="tmpht")
                tmplt = iop.tile([P, TC * C], U8, tag="tmplt")
                pett = iop.tile([P, TC * C], U16, tag="pett")
                dybt = iop.tile([P, TC * F], U16, tag="dybt")
                dyet = iop.tile([P, TC * F], U16, tag="dyet")
                nc.gpsimd.dma_start(out=prct[:, :],
                                    in_=prcd[:, c0 * C:(c0 + TC) * C])
                nc.gpsimd.dma_start(out=tmpht[:, :],
                                    in_=tmphd[:, c0 * C:(c0 + TC) * C])
                nc.gpsimd.dma_start(out=tmplt[:, :],
                                    in_=tmpld[:, c0 * C:(c0 + TC) * C])
                nc.gpsimd.dma_start(out=pett[:, :],
                                    in_=petd[:, c0 * C:(c0 + TC) * C])
                nc.gpsimd.dma_start(out=dybt[:, :],
                                    in_=dybd[:, c0 * F:(c0 + TC) * F])
                nc.gpsimd.dma_start(out=dyet[:, :],
                                    in_=dyed[:, c0 * F:(c0 + TC) * F])

                # ---- bulk derive on POOL/DVE ----
                raint = der.tile([P, TC * F], f32, tag="raint")
                snowt = der.tile([P, TC * F], f32, tag="snowt")
                mpt = der.tile([P, TC * F], f32, tag="mpt")
                rpt = der.tile([P, TC * F], f32, tag="rpt")
                m1t = der.tile([P, TC * F], f32, tag="m1t")
                dybf = der.tile([P, TC * F], f32, tag="dybf")
                dyef = der.tile([P, TC * F], f32, tag="dyef")
                comb = der.tile([P, TC * C], f32, tag="comb")

                def fb3(tile_ap):  # [P, TC*C] -> [P,TC,C]
                    return tile_ap.rearrange("p (t c) -> p t c", t=TC)

                def dv(tile_ap, m):  # [P, TC*F] -> m-slice [P,TC,C]
                    return tile_ap.rearrange(
                        "p (t m c) -> p t m c", t=TC, m=NMUL)[:, :, m, :]

                def sbcm(name, m):  # static m-slice -> bc [P,TC,C]
                    return spm(name, m).unsqueeze(1).broadcast_to([P, TC, C])

                # reassemble 24-bit tmean code: v24 = hi*256 + lo (exact f32)
                V.scalar_tensor_tensor(comb[:, :], tmpht[:, :], 256.0,
                                       tmplt[:, :], OP.mult, OP.add)
                # 2D sem-absorbers: 3D ops below may carry at most 1 wait
                V.tensor_copy(m1t[:, 0:1], comb[:, 0:1])
                V.tensor_copy(raint[:, 0:1], comb[:, 0:1])
                V.tensor_copy(snowt[:, 0:1], comb[:, 0:1])
                V.tensor_copy(mpt[:, 0:1], cst_t[:, 0:1])
                V.tensor_copy(rpt[:, 0:1], comb[:, 0:1])
                T3 = fb3(comb[:, :])
                P3 = fb3(prct[:, :])
                for m in range(NMUL):
                    # m1t = (v24*scl) - (parTT - LO) == T_degC - parTT
                    V.scalar_tensor_tensor(dv(m1t[:, :], m), T3, SCL_T24,
                                           sbcm("TTlo", m), OP.mult, OP.subtract)
                    # mask on the exact integer code (host-nudged)
                    V.tensor_tensor(dv(raint[:, :], m), T3,
                                    sbcm("TTu24", m), OP.is_ge)
                    # rain/snow kept in u16 prcp units; scaled at use sites
                    V.tensor_tensor(dv(raint[:, :], m), dv(raint[:, :], m),
                                    P3, OP.mult)
                    V.tensor_tensor(dv(snowt[:, :], m), P3,
                                    dv(raint[:, :], m), OP.subtract)
                for m in range(NMUL):
                    V.tensor_tensor(dv(mpt[:, :], m), dv(m1t[:, :], m),
                                    sbcm("parCFMAX", m), OP.mult)
                V.tensor_scalar_max(mpt[:, :], mpt[:, :], 0.0)
                V.tensor_scalar_min(m1t[:, :], m1t[:, :], 0.0)
                for m in range(NMUL):
                    V.tensor_tensor(dv(rpt[:, :], m), dv(m1t[:, :], m),
                                    sbcm("ncc", m), OP.mult)
                # dequant + scale dynamic params u16 -> f32
                V.tensor_scalar(dybf[:, :], dybt[:, :], 5.0 * SCL_DY, 1.0,
                                OP.mult, OP.add)
                V.tensor_scalar(dyef[:, :], dyet[:, :], 4.7 * SCL_DY, 0.3,
                                OP.mult, OP.add)

                # ---- sequential scan ----
                for t in range(TC):
                    SNOW_t = snowt[:, t * F:(t + 1) * F]
                    mp_t = mpt[:, t * F:(t + 1) * F]
                    rp_t = rpt[:, t * F:(t + 1) * F]
                    RAIN_t = raint[:, t * F:(t + 1) * F]
                    beta_t = dybf[:, t * F:(t + 1) * F]
                    betaet_t = dyef[:, t * F:(t + 1) * F]

                    # snow bucket (SNOW_t/RAIN_t are in u16 prcp units)
                    V.scalar_tensor_tensor(s_sp1[:, :], SNOW_t, SCL_PRC,
                                           SP_[:, :], OP.mult, OP.add)
                    V.tensor_tensor(s_melt[:, :], mp_t, s_sp1[:, :], OP.min)
                    V.tensor_tensor(s_mw1[:, :], MW[:, :], s_melt[:, :], OP.add)
                    V.tensor_tensor(s_sp1[:, :], s_sp1[:, :], s_melt[:, :], OP.subtract)
                    V.tensor_tensor(s_rfz[:, :], rp_t, s_mw1[:, :], OP.min)
                    V.tensor_tensor(SP_[:, :], s_sp1[:, :], s_rfz[:, :], OP.add)
                    V.tensor_tensor(s_mw2[:, :], s_mw1[:, :], s_rfz[:, :], OP.subtract)
                    V.tensor_tensor(s_cw[:, :], sp("parCWH"), SP_[:, :], OP.mult)
                    V.tensor_tensor(s_t9[:, :], s_mw2[:, :], s_cw[:, :], OP.subtract)
                    A.activation(s_tos[:, :], s_t9[:, :], AF.Relu)
                    V.tensor_tensor(MW[:, :], s_mw2[:, :], s_tos[:, :], OP.subtract)
                    V.scalar_tensor_tensor(s_rts[:, :], RAIN_t, SCL_PRC,
                                           s_tos[:, :], OP.mult, OP.add)

                    # soil bucket
                    V.tensor_tensor(s_x[:, :], SM[:, :], sp("invFC"), OP.mult)
                    A.activation(s_lx[:, :], s_x[:, :], AF.Ln)
                    V.tensor_tensor(s_e[:, :], beta_t, s_lx[:, :], OP.mult)
                    V.tensor_scalar_min(s_e[:, :], s_e[:, :], 0.0)
                    A.activation(s_pw[:, :], s_e[:, :], AF.Exp)
                    V.tensor_tensor(s_rch[:, :], s_rts[:, :], s_pw[:, :], OP.mult)
                    V.tensor_tensor(s_d1[:, :], s_rts[:, :], s_rch[:, :], OP.subtract)
                    V.tensor_tensor(s_sm1[:, :], SM[:, :], s_d1[:, :], OP.add)
                    V.tensor_tensor(s_sm2[:, :], s_sm1[:, :], sp("parFC"), OP.min)
                    V.tensor_tensor(s_exs[:, :], s_sm1[:, :], s_sm2[:, :], OP.subtract)
                    V.tensor_tensor(s_y[:, :], s_sm2[:, :], sp("invLPFC"), OP.mult)
                    A.activation(s_ly[:, :], s_y[:, :], AF.Ln)
                    V.scalar_tensor_tensor(s_f2[:, :], s_ly[:, :], 0.0,
                                           betaet_t, OP.min, OP.mult)
                    A.activation(s_ev[:, :], s_f2[:, :], AF.Exp)
                    for m in range(NMUL):
                        V.scalar_tensor_tensor(s_pe[:, m * C:(m + 1) * C],
                                               pett[:, t * C:(t + 1) * C],
                                               SCL_PET,
                                               s_ev[:, m * C:(m + 1) * C],
                                               OP.mult, OP.mult)
                    V.tensor_tensor(s_eta[:, :], s_sm2[:, :], s_pe[:, :], OP.min)
                    V.tensor_tensor(s_sm3[:, :], s_sm2[:, :], s_eta[:, :], OP.subtract)
                    V.tensor_scalar_max(s_sm3[:, :], s_sm3[:, :], NZ)
                    # capillary
                    V.tensor_tensor(s_z[:, :], s_sm3[:, :], sp("invFC"), OP.mult)
                    V.tensor_scalar(s_zm[:, :], s_z[:, :], 1.0, -1.0, OP.min, OP.mult)
                    V.tensor_tensor(s_u1[:, :], SLZ[:, :], sp("parC"), OP.mult)
                    V.scalar_tensor_tensor(s_cap[:, :], s_zm[:, :], 1.0,
                                           s_u1[:, :], OP.add, OP.mult)
                    V.tensor_tensor(SM[:, :], s_sm3[:, :], s_cap[:, :], OP.add)
                    V.tensor_tensor(SLZ[:, :], SLZ[:, :], s_cap[:, :], OP.subtract)
                    V.tensor_scalar_max(SLZ[:, :], SLZ[:, :], NZ)

                    # groundwater
                    G.tensor_tensor(s_su1[:, :], SUZ[:, :], s_rch[:, :], OP.add)
                    G.tensor_tensor(s_su1[:, :], s_su1[:, :], s_exs[:, :], OP.add)
                    G.tensor_tensor(s_su2[:, :], s_su1[:, :], sp("parPERC"), OP.subtract)
                    A.activation(s_suz2[:, :], s_su2[:, :], AF.Relu)
                    G.tensor_tensor(s_perc[:, :], s_su1[:, :], s_suz2[:, :], OP.subtract)
                    G.tensor_tensor(s_q0a[:, :], s_suz2[:, :], sp("parUZL"), OP.subtract)
                    V.scalar_tensor_tensor(s_q0[:, :], s_q0a[:, :], 0.0,
                                           sp("parK0"), OP.max, OP.mult)
                    G.tensor_tensor(s_suz2[:, :], s_suz2[:, :], s_q0[:, :], OP.subtract)
                    G.tensor_tensor(s_q1[:, :], sp("parK1"), s_suz2[:, :], OP.mult)
                    G.tensor_tensor(SUZ[:, :], s_suz2[:, :], s_q1[:, :], OP.subtract)
                    G.tensor_tensor(SLZ[:, :], SLZ[:, :], s_perc[:, :], OP.add)
                    G.tensor_tensor(s_gw2[:, :], SLZ[:, :], sp("rtclip"), OP.subtract)
                    V.scalar_tensor_tensor(s_q2[:, :], s_gw2[:, :], 0.0,
                                           sp("parK2"), OP.max, OP.mult)
                    V.scalar_tensor_tensor(SLZ[:, :], s_gw2[:, :], 0.0,
                                           s_q2[:, :], OP.max, OP.subtract)
                    # Qt and nmul-sum (mean folded into UH weights)
                    G.tensor_tensor(s_qa[:, :], s_q0[:, :], s_q1[:, :], OP.add)
                    G.tensor_tensor(s_qa[:, :], s_qa[:, :], s_q2[:, :], OP.add)
                    tq = c0 + t
                    G.tensor_tensor(Qbuf[:, tq * C:(tq + 1) * C],
                                    s_qa[:, 0:C], s_qa[:, C:F], OP.add)

            # ---------------- UH routing ----------------
            # flow[t] = sum_k wn[k] * Q[t-k]; DVE handles t in [0,TS), POOL the rest
            TS = (T * 7) // 10
            rtmp = big.tile([P, T * C], f32)
            flow16 = big.tile([P, T * C], F16)

            def conv_range(eng, t_lo, t_hi):
                for k in range(LENF):
                    o_lo = max(t_lo, k)
                    n = t_hi - o_lo
                    if n <= 0:
                        continue
                    wk_bc = wnk(k).unsqueeze(1).broadcast_to([P, n, C])
                    qsh = Qbuf[:, (o_lo - k) * C:(o_lo - k + n) * C] \
                        .rearrange("p (t c) -> p t c", t=n)
                    out = FLOW[:, o_lo * C:(o_lo + n) * C] \
                        .rearrange("p (t c) -> p t c", t=n)
                    if k == 0:
                        eng.tensor_tensor(out, wk_bc, qsh, OP.mult)
                    else:
                        tmp = rtmp[:, o_lo * C:(o_lo + n) * C] \
                            .rearrange("p (t c) -> p t c", t=n)
                        eng.tensor_tensor(tmp, wk_bc, qsh, OP.mult)
                        eng.tensor_tensor(out, out, tmp, OP.add)

            conv_range(V, 0, TS)
            conv_range(G, TS, T)
            # convert to fp16 for the wire, split across engines
            TH = T // 2
            V.tensor_copy(flow16[:, :TH * C], FLOW[:, :TH * C])
            G.tensor_copy(flow16[:, TH * C:], FLOW[:, TH * C:])

            nc.gpsimd.dma_start(out=flowd[:, :], in_=flow16[:, :])
    return nc


def _prep_forc_u16(src_TG, inv_scl, out_u16, scr):
    # round-to-nearest u16 quantization (float->uint assignment truncates,
    # so add 0.5 first) fused with the [T,G] -> [KP, T*C] layout transpose
    T, K = T_TOTAL, NCORES
    np.multiply(src_TG, np.float32(inv_scl), out=scr)
    scr += np.float32(0.5)
    out_u16.reshape(K, P, T, C)[...] = scr.reshape(T, K, P, C).transpose(1, 2, 0, 3)


def _tt_of(params_stat):
    st = params_stat[:, :14 * NMUL].reshape(N_GRID, 14, NMUL)
    return st[:, 7] * np.float32(5.0) + np.float32(-2.5)      # [G, NMUL]


def _prep_tmp24(x_phy, TT, tmph, tmpl, scr, scrh):
    """24-bit tmean code v24, nudged so (v24 >= TTu24) == (T32 >= parTT)
    exactly; split into u16 high and u8 low halves. All v24 values stay
    below 2^24, so every quantity here is exact in f32."""
    T, K = T_TOTAL, NCORES
    t32 = x_phy[:, :, 1]
    np.subtract(t32, np.float32(LO_TMP), out=scr)
    scr *= np.float32(1.0 / 30.0 * 65535.0 * 256.0)
    scr += np.float32(0.5)
    np.floor(scr, out=scr)                                    # v24
    TTu24 = (TT - np.float32(LO_TMP)) * np.float32(1.0 / SCL_T24)
    for _ in range(4):
        bad_up = None
        bad_dn = None
        for m in range(NMUL):
            m32 = t32 >= TT[None, :, m]
            mq = scr >= TTu24[None, :, m]
            up = m32 & ~mq
            dn = ~m32 & mq
            bad_up = up if bad_up is None else (bad_up | up)
            bad_dn = dn if bad_dn is None else (bad_dn | dn)
        if not (bad_up.any() or bad_dn.any()):
            break
        scr[bad_up] += np.float32(1.0)
        scr[bad_dn] -= np.float32(1.0)
    np.multiply(scr, np.float32(1.0 / 256.0), out=scrh)
    np.floor(scrh, out=scrh)                                  # hi
    tmph.reshape(K, P, T, C)[...] = scrh.reshape(T, K, P, C).transpose(1, 2, 0, 3)
    scrh *= np.float32(-256.0)
    scr += scrh                                               # lo = v24 - hi*256
    tmpl.reshape(K, P, T, C)[...] = scr.reshape(T, K, P, C).transpose(1, 2, 0, 3)


def _prep_dy(params_dy, j, out_u16, scr2):
    # dynamic params: u16 in [0,1]; j = 0 (parBETA) or 1 (parBETAET)
    T, K = T_TOTAL, NCORES
    d = params_dy.reshape(T, N_GRID, 2, NMUL)
    np.multiply(d[:, :, j, :], np.float32(65535.0), out=scr2)
    scr2 += np.float32(0.5)
    out_u16.reshape(K, P, T, NMUL, C)[...] = \
        scr2.reshape(T, K, P, C, NMUL).transpose(1, 2, 0, 4, 3)


def _prep_cst(ac_all, params_stat, TT, cst):
    G, K = N_GRID, NCORES
    st = params_stat[:, :14 * NMUL].reshape(G, 14, NMUL)

    # statics, prescaled in f32 (matches on-device math of the baseline)
    vals = {}
    for i, name in enumerate(STAT_NAMES):
        lo, hi = BOUNDS[name]
        vals[name] = st[:, i] * np.float32(hi - lo) + np.float32(lo)
    invFC = np.float32(1.0) / vals["parFC"]
    invLPFC = np.float32(1.0) / (vals["parLP"] * vals["parFC"])
    ncc = -(vals["parCFR"] * vals["parCFMAX"])
    acq = np.clip(np.float32(1.0) - ac_all[:, None].astype(np.float32)
                  / (vals["parAC"] + np.float32(NZ)), 0.0, 1.0).astype(np.float32)
    rtclip = vals["parRT"] * acq
    table = {"TTlo": TT - np.float32(LO_TMP),
             "TTu24": (TT - np.float32(LO_TMP)) * np.float32(1.0 / SCL_T24),
             "parCFMAX": vals["parCFMAX"], "parCWH": vals["parCWH"],
             "parFC": vals["parFC"], "invFC": invFC, "invLPFC": invLPFC,
             "ncc": ncc, "parC": vals["parC"], "parPERC": vals["parPERC"],
             "parUZL": vals["parUZL"], "parK0": vals["parK0"],
             "parK1": vals["parK1"], "parK2": vals["parK2"], "rtclip": rtclip}
    for name in CST_ORDER:
        i = SIDX[name]
        cst[:, i * F:(i + 1) * F].reshape(K, P, NMUL, C)[...] = \
            table[name].reshape(K, P, C, NMUL).transpose(0, 1, 3, 2)

    # UH weights (gammaln and theta^-a cancel under normalization);
    # fold the nmul-mean (x0.5) in
    rta = params_stat[:, 14 * NMUL].astype(np.float32)
    rtb = params_stat[:, 14 * NMUL + 1].astype(np.float32)
    a = np.maximum(rta * np.float32(ROUT_A[1]), 0) + np.float32(0.1)
    th = np.maximum(rtb * np.float32(ROUT_B[1]), 0) + np.float32(0.5)
    tk = (np.arange(LENF) + 0.5).astype(np.float32)
    w = np.exp((a - np.float32(1.0))[None, :] * np.log(tk)[:, None]
               - tk[:, None] / th[None, :]).astype(np.float32)
    w /= w.sum(0)
    w *= np.float32(0.5)
    cst[:, len(CST_ORDER) * F:].reshape(K, P, LENF, C)[...] = \
        w.reshape(LENF, K, P, C).transpose(1, 2, 0, 3)


_RT = {}


def _get_rt():
    if _RT:
        return _RT
    import jax
    from jax.sharding import Mesh, PartitionSpec, NamedSharding

    nc = bacc.Bacc()
    _build(nc)
    nc.compile()

    from concourse import bass2jax
    bass2jax.install_neuronx_cc_hook()

    partition_name = nc.partition_id_tensor.name if nc.partition_id_tensor else None
    in_names, out_names, out_avals = [], [], []
    for alloc in nc.m.functions[0].allocations:
        if not isinstance(alloc, mybir.MemoryLocationSet):
            continue
        name = alloc.memorylocations[0].name
        if alloc.kind == "ExternalInput":
            if name != partition_name:
                in_names.append(name)
        elif alloc.kind == "ExternalOutput":
            out_names.append(name)
            out_avals.append(jax.core.ShapedArray(
                tuple(alloc.tensor_shape), mybir.dt.np(alloc.dtype)))
    assert in_names == ["cst", "prc", "pet", "tmph", "tmpl", "dyb", "dye"] \
        and out_names == ["flow"], (in_names, out_names)
    n_params = len(in_names)
    in_names_all = in_names + out_names
    if partition_name is not None:
        in_names_all = in_names_all + [partition_name]

    def _body(*args):
        operands = list(args)
        if partition_name is not None:
            operands.append(bass2jax.partition_id_tensor())
        outs = bass2jax._bass_exec_p.bind(
            *operands, out_avals=tuple(out_avals), in_names=tuple(in_names_all),
            out_names=tuple(out_names), lowering_input_output_aliases=(),
            sim_require_finite=True, sim_require_nnan=True, nc=nc)
        return tuple(outs)

    devices = jax.devices()[:NCORES]
    assert len(devices) == NCORES
    mesh = Mesh(np.asarray(devices), ("core",))
    spec = PartitionSpec("core")
    fn = jax.jit(
        jax.shard_map(_body, mesh=mesh, in_specs=(spec,) * (n_params + 1),
                      out_specs=(spec,)),
        donate_argnums=(n_params,), keep_unused=True)
    from concurrent.futures import ThreadPoolExecutor
    _RT.update(dict(jax=jax, fn=fn, sh=NamedSharding(mesh, spec),
                    donate=None, pool=ThreadPoolExecutor(max_workers=8)))
    return _RT


def kernel(x_phy, ac_all, elev_all, params_dy, params_stat, _trace=False):
    import time
    t0 = time.time()
    rt = _get_rt()
    jax = rt["jax"]
    x_phy = np.asarray(x_phy, dtype=np.float32)
    ac_all = np.asarray(ac_all, dtype=np.float32)
    params_dy = np.asarray(params_dy, dtype=np.float32)
    params_stat = np.asarray(params_stat, dtype=np.float32)
    T = x_phy.shape[0]
    assert T == T_TOTAL, f"kernel built for T={T_TOTAL}, got {T}"
    t1 = time.time()

    sh = rt["sh"]
    bufs = rt.get("bufs")
    if bufs is None:
        KP = NCORES * P
        bufs = {"prc": np.empty((KP, T * C), np.uint16),
                "pet": np.empty((KP, T * C), np.uint16),
                "tmph": np.empty((KP, T * C), np.uint16),
                "tmpl": np.empty((KP, T * C), np.uint8),
                "dyb": np.empty((KP, T * F), np.uint16),
                "dye": np.empty((KP, T * F), np.uint16),
                "cst": np.empty((KP, NCST), np.float32),
                "scr": np.empty((T, N_GRID), np.float32),
                "scrh": np.empty((T, N_GRID), np.float32),
                "scr2": np.empty((T, N_GRID, NMUL), np.float32)}
        rt["bufs"] = bufs

    # staged prep -> put. Puts are issued from worker threads: device_put
    # can block when too many transfers are outstanding, and a blocked put
    # must not stall the main thread's prep of the remaining regions.
    pool = rt["pool"]
    futs = {}
    TT = _tt_of(params_stat)
    _prep_cst(ac_all, params_stat, TT, bufs["cst"])
    futs["cst"] = pool.submit(jax.device_put, bufs["cst"], sh)
    _prep_forc_u16(x_phy[:, :, 0], 1.0 / SCL_PRC, bufs["prc"], bufs["scr"])
    futs["prc"] = pool.submit(jax.device_put, bufs["prc"], sh)
    _prep_forc_u16(x_phy[:, :, 2], 1.0 / SCL_PET, bufs["pet"], bufs["scr"])
    futs["pet"] = pool.submit(jax.device_put, bufs["pet"], sh)
    _prep_tmp24(x_phy, TT, bufs["tmph"], bufs["tmpl"], bufs["scr"], bufs["scrh"])
    futs["tmph"] = pool.submit(jax.device_put, bufs["tmph"], sh)
    futs["tmpl"] = pool.submit(jax.device_put, bufs["tmpl"], sh)
    _prep_dy(params_dy, 0, bufs["dyb"], bufs["scr2"])
    futs["dyb"] = pool.submit(jax.device_put, bufs["dyb"], sh)
    _prep_dy(params_dy, 1, bufs["dye"], bufs["scr2"])
    futs["dye"] = pool.submit(jax.device_put, bufs["dye"], sh)
    don = rt["donate"]
    if don is None:
        don = jax.device_put(np.zeros((NCORES * P, T * C), np.float16), sh)
    t3 = time.time()

    out, = rt["fn"](futs["cst"].result(), futs["prc"].result(),
                    futs["pet"].result(), futs["tmph"].result(),
                    futs["tmpl"].result(), futs["dyb"].result(),
                    futs["dye"].result(), don)
    # per-shard fetch in worker threads, with the layout transpose for shard
    # k overlapping the tunnel fetch of shards k+1..: hides most of post
    out.block_until_ready()
    t4 = time.time()
    full = np.empty((T, N_GRID), np.float32)
    fullv = full.reshape(T, NCORES, P, C)
    shards = out.addressable_shards

    def _fetch(i):
        sh_ = shards[i]
        return (sh_.index[0].start or 0) // P, np.asarray(sh_.data)

    from concurrent.futures import as_completed
    fetch_futs = [pool.submit(_fetch, i) for i in range(len(shards))]
    for fu in as_completed(fetch_futs):
        k, arr = fu.result()
        fullv[:, k] = arr.reshape(P, T, C).transpose(1, 0, 2)
    rt["donate"] = out             # reuse device buffer as next call's output alloc
    t5 = time.time()
    if _TIMING:
        print(f"[kernel] setup {t1-t0:.3f}s prep+put {t3-t1:.3f}s "
              f"xfer+exec {t4-t3:.3f}s fetch+post {t5-t4:.3f}s total {t5-t0:.3f}s",
              flush=True)
    return full[..., None]
